# revision 20
# baseline (speedup 1.0000x reference)
"""Trainium2 Bass kernel for nn_DarcyResidual (P=256, B=128, 8 NeuronCores).

Math (reference):
    a = (x0 + 1.5) / 0.2,  p = (x1 + 0.9) / 115
    residual = -a*(p_d00 + p_d11) - a_d0*p_d0 - a_d1*p_d1 - 1
2nd-order central differences inside, 2nd-order one-sided at borders,
h = 1/256 on both axes.

Folded form computed here (G = 5/(460 h^2)):
    residual = -G * [ (X0 + 1.5)*U4 + S1*R1 + C1a*C1p ] - 1
      U4  = 4*(rowD2raw(X1) + colD2raw(X1))   (raw h^2-scaled 2nd diffs)
      R1  = rowD1raw(X1), S1 = rowD1raw(X0)   (raw 2h-scaled 1st diffs)
      C1p = colD1raw(X1), C1a = colD1raw(X0)

Layout per core (16 images): SBUF [partition = row-within-128-block,
free = (row-block k:2, image b, col j:256)].  Row (d0) derivatives are
TensorE matmuls with banded stencil matrices in fp32r (boundary rows are
rows of the matrices; the -2I of the column stencil is folded in as
W_R2 = 4*(D2 - 2I)).  Column (d1) stencils are shifted-AP DVE ops.  The
two small gradient-product terms are written as bf16 and summed in PSUM
via identity-matmul accumulation.  Border columns j=0,255 get their own
small one-sided pipeline.  ScalarE does PSUM evacuation + final affine.
"""

import numpy as np

P = 256
B = 128
NCORES = 8
BPC = B // NCORES          # images per core = 16
CHUNKS = 8
BCH = BPC // CHUNKS        # images per chunk = 2
FCH = 2 * BCH * P          # chunk free size = 2048
GAMMA = 5.0 * 65536.0 / 460.0

_cache = {}


def _weights():
    """[128, 12, 128] stacked lhsT blocks for the row-stencil matmuls."""
    D1 = np.zeros((P, P), dtype=np.float64)
    for i in range(1, P - 1):
        D1[i, i - 1] = -1.0
        D1[i, i + 1] = 1.0
    D1[0, 0:3] = [-3.0, 4.0, -1.0]
    D1[P - 1, P - 3:P] = [1.0, -4.0, 3.0]

    D2 = np.zeros((P, P), dtype=np.float64)
    for i in range(1, P - 1):
        D2[i, i - 1] = 1.0
        D2[i, i] = -2.0
        D2[i, i + 1] = 1.0
    D2[0, 0:4] = [2.0, -5.0, 4.0, -1.0]
    D2[P - 1, P - 4:P] = [-1.0, 4.0, -5.0, 2.0]

    WR2 = 4.0 * (D2 - 2.0 * np.eye(P))   # main (col -2I folded here)
    WR2E = 4.0 * D2                      # edge (col stencil complete)

    wts = np.zeros((128, 13, 128), dtype=np.float32)
    wts[:, 12, :] = 4.0 * np.eye(128)
    for m in range(2):
        for kb in range(2):
            i = m * 2 + kb
            blk = lambda W: W[m * 128:(m + 1) * 128, kb * 128:(kb + 1) * 128].T
            wts[:, i, :] = blk(D1)
            wts[:, 4 + i, :] = blk(WR2)
            wts[:, 8 + i, :] = blk(WR2E)
    return wts


def _build_program():
    from concourse import bacc
    import concourse.mybir as mybir
    from concourse.tile import TileContext
    import ml_dtypes

    f32 = mybir.dt.float32
    f32r = mybir.dt.float32r
    bf16 = mybir.dt.bfloat16
    ADD = mybir.AluOpType.add
    SUB = mybir.AluOpType.subtract
    MUL = mybir.AluOpType.mult
    COPY = mybir.ActivationFunctionType.Copy

    nc = bacc.Bacc("TRN2", target_bir_lowering=False, debug=False,
                   num_devices=NCORES)
    xin = nc.dram_tensor("xin", [128, 2, 2, BPC, P], f32r, kind="ExternalInput")
    xe = nc.dram_tensor("xe", [128, 2, 2, BPC, 8], f32r, kind="ExternalInput")
    wts = nc.dram_tensor("wts", [128, 13, 128], f32r, kind="ExternalInput")
    xb = nc.dram_tensor("xb", [128, 2, 2, BPC, P], bf16, kind="ExternalInput")
    ident = nc.dram_tensor("ident", [128, 128], bf16, kind="ExternalInput")
    wtbd = nc.dram_tensor("wtbd", [128, 4, 128], bf16, kind="ExternalInput")
    yout = nc.dram_tensor("yout", [128, 2, BPC, P], f32, kind="ExternalOutput")

    with TileContext(nc) as tc:
        with (
            tc.tile_pool(name="const", bufs=1) as cpool,
            tc.tile_pool(name="edge", bufs=1) as epool,
            tc.tile_pool(name="work", bufs=2) as pool,
            tc.tile_pool(name="psum", bufs=2, space="PSUM") as pp,
        ):
            wt = cpool.tile([128, 13, 128], f32r)
            nc.sync.dma_start(out=wt[:], in_=wts[:])
            ibf = cpool.tile([128, 128], bf16)
            nc.sync.dma_start(out=ibf[:], in_=ident[:])
            wtb = cpool.tile([128, 4, 128], bf16)
            nc.sync.dma_start(out=wtb[:], in_=wtbd[:])

            def Wb(i):
                return wtb[:, i, :]

            def W(i):
                return wt[:, i, :]

            stt = nc.vector.scalar_tensor_tensor

            # ------------- edge pipeline (output cols j=0 and j=255) -------
            # (emitted after chunk 0's body so chunk-0 stencils start first)
            X0e = epool.tile([128, 2, BPC, 8], f32r)
            X1e = epool.tile([128, 2, BPC, 8], f32r)
            nc.sync.dma_start(out=X0e[:], in_=xe[:, 0])
            nc.sync.dma_start(out=X1e[:], in_=xe[:, 1])

            X0ef = X0e.rearrange("p k b c -> p (k b c)")
            X1ef = X1e.rearrange("p k b c -> p (k b c)")
            # fp32 views, [128, 32, 8]
            E1 = X1e.bitcast(f32).rearrange("p k b c -> p (k b) c")
            E0 = X0e.bitcast(f32).rearrange("p k b c -> p (k b) c")

            def et(name, d=2):
                return epool.tile([128, 2 * BPC, d], f32, name=name, tag=name)

            if True:
                R2e = pp.tile([128, 2, BPC, 8], f32, tag="r2")
                R1e = pp.tile([128, 2, BPC, 8], f32, tag="r1")
                S1e = pp.tile([128, 2, BPC, 8], f32, tag="s1")
                R2ef = R2e.rearrange("p k b c -> p (k b c)")
                R1ef = R1e.rearrange("p k b c -> p (k b c)")
                S1ef = S1e.rearrange("p k b c -> p (k b c)")
                for m in range(2):
                    osl = slice(m * 128, (m + 1) * 128)
                    for kb in range(2):
                        isl = slice(kb * 128, (kb + 1) * 128)
                        st, sp = kb == 0, kb == 1
                        nc.tensor.matmul(R1ef[:, osl], W(m * 2 + kb),
                                         X1ef[:, isl], start=st, stop=sp)
                        nc.tensor.matmul(S1ef[:, osl], W(m * 2 + kb),
                                         X0ef[:, isl], start=st, stop=sp)
                        nc.tensor.matmul(R2ef[:, osl], W(8 + m * 2 + kb),
                                         X1ef[:, isl], start=st, stop=sp)

                # paired forward/mirrored diffs: half 0 = j=0 side (fwd),
                # half 1 = j=255 side (also forward-oriented: f7-f6 etc.)
                a1, b1, c1 = et("a1"), et("b1"), et("c1")
                a0, b0 = et("a0"), et("b0")
                nc.vector.tensor_sub(a1[:], E1[:, :, 1:8:6], E1[:, :, 0:7:6])
                nc.vector.tensor_sub(b1[:], E1[:, :, 2:7:4], E1[:, :, 1:6:4])
                nc.vector.tensor_sub(c1[:], E1[:, :, 3:6:2], E1[:, :, 2:5:2])
                nc.vector.tensor_sub(a0[:], E0[:, :, 1:8:6], E0[:, :, 0:7:6])
                nc.vector.tensor_sub(b0[:], E0[:, :, 2:7:4], E0[:, :, 1:6:4])

                # one-sided raw stencils (Z sign flips on the mirror half)
                q, Z = et("q"), et("Z")
                C1pe, C1ae = et("C1pe"), et("C1ae")
                stt(q[:], b1[:], 3.0, c1[:], MUL, SUB)      # 3b - c
                stt(Z[:], a1[:], -2.0, q[:], MUL, ADD)      # -2a + 3b - c
                stt(C1pe[:], a1[:], 3.0, b1[:], MUL, SUB)   # 3a - b
                stt(C1ae[:], a0[:], 3.0, b0[:], MUL, SUB)

                RP2 = R2e.rearrange("p k b c -> p (k b) c")
                RP1 = R1e.rearrange("p k b c -> p (k b) c")
                U4e, tme, t2e = et("U4e"), et("tme"), et("t2e")
                stt(U4e[:, :, 0:1], Z[:, :, 0:1], 4.0, RP2[:, :, 0:1], MUL, ADD)
                stt(U4e[:, :, 1:2], Z[:, :, 1:2], -4.0, RP2[:, :, 7:8], MUL, ADD)

                Scpe = epool.tile([128, 2, BPC, 8], f32)
                nc.scalar.copy(out=Scpe.rearrange("p k b c -> p (k b c)"),
                               in_=S1ef[:])
                SP = Scpe.rearrange("p k b c -> p (k b) c")

                stt(tme[:], E0[:, :, 0:8:7], 1.5, U4e[:], ADD, MUL)
                nc.vector.tensor_mul(t2e[:], SP[:, :, 0:8:7], RP1[:, :, 0:8:7])
                nc.vector.tensor_add(tme[:], tme[:], t2e[:])
                nc.vector.tensor_mul(C1ae[:], C1ae[:], C1pe[:])  # t3e in-place
                nc.vector.tensor_add(tme[:], tme[:], C1ae[:])
                rese = epool.tile([128, 2 * BPC, 2], f32)
                nc.scalar.activation(rese[:], tme[:], COPY,
                                     bias=-1.0, scale=-GAMMA)

            # ------------- main pipeline, 4 chunks of 4 images -------------
            if True:
                for c in range(CHUNKS):
                    b0c = c * BCH
                    XbC = pool.tile([128, 2, 2, BCH, P], bf16, tag="xbc",
                                    bufs=3)
                    nc.sync.dma_start(out=XbC[:], in_=xb[:, :, :, b0c:b0c + BCH, :])
                    X0c = pool.tile([128, 2, BCH, P], f32r, tag="x0", bufs=3)
                    # X1 padded by one col each side so the +-1-shifted
                    # identity-matmul rhs views stay in bounds at full N
                    X1p = pool.tile([128, FCH + 2], f32r, tag="x1", bufs=3)
                    nc.sync.dma_start(
                        out=X1p[:, 1:FCH + 1].rearrange(
                            "p (k b j) -> p k b j", k=2, b=BCH),
                        in_=xin[:, 1, :, b0c:b0c + BCH, :])
                    nc.sync.dma_start(out=X0c[:], in_=xin[:, 0, :, b0c:b0c + BCH, :])
                    XbCf = XbC.rearrange("p c k b j -> p (c k b j)")
                    Xb0f = XbCf[:, 0:FCH]
                    Xb1f = XbCf[:, FCH:2 * FCH]
                    X0rf = X0c.rearrange("p k b j -> p (k b j)")
                    X1rf = X1p[:, 1:FCH + 1]
                    X0f = X0c.bitcast(f32).rearrange("p k b j -> p (k b j)")
                    X1pf = X1p.bitcast(f32)
                    X1f = X1pf[:, 1:FCH + 1]
                    # "sh" tiles hold values for column index t+1 at slot t,
                    # keeping every bf16 operand 4-byte aligned (2x mode).
                    C1p = pool.tile([128, FCH], bf16, tag="c1p", bufs=3)
                    C1a = pool.tile([128, FCH], bf16, tag="c1a", bufs=3)
                    scp = pool.tile([128, FCH], f32, tag="scp", bufs=3)
                    rcp = pool.tile([128, FCH], f32, tag="rcp", bufs=3)
                    t2b = pool.tile([128, FCH], bf16, tag="t2b", bufs=3)
                    t3b = pool.tile([128, FCH], bf16, tag="t3b", bufs=3)
                    tm = pool.tile([128, 2, BCH, P], f32, tag="tm", bufs=3)
                    tmf = tm.rearrange("p k b j -> p (k b j)")

                    # column stencils (interior; border cols handled above)
                    nc.vector.tensor_sub(C1p[:, 0:FCH - 2], Xb1f[:, 2:FCH],
                                         Xb1f[:, 0:FCH - 2])
                    nc.vector.tensor_sub(C1a[:, 0:FCH - 2], Xb0f[:, 2:FCH],
                                         Xb0f[:, 0:FCH - 2])
                    nc.vector.tensor_mul(t3b[:], C1a[:], C1p[:])

                    for m in range(2):
                        NBP = BCH // 2
                        R1s = [pp.tile([128, 2 * P], f32, name=f"r1_{c}_{m}_{bp}",
                                       tag="r1") for bp in range(NBP)]
                        S1s = [pp.tile([128, 2 * P], f32, name=f"s1_{c}_{m}_{bp}",
                                       tag="s1") for bp in range(NBP)]
                        U4s = [pp.tile([128, 2 * P], f32, name=f"r2_{c}_{m}_{bp}",
                                       tag="r2") for bp in range(NBP)]
                        for kb in range(2):
                            st, sp = kb == 0, kb == 1
                            isls = [slice(kb * (BCH * P) + bp * (2 * P),
                                          kb * (BCH * P) + (bp + 1) * (2 * P))
                                    for bp in range(NBP)]
                            for bp in range(NBP):
                                nc.tensor.matmul(R1s[bp][:], Wb(m * 2 + kb),
                                                 Xb1f[:, isls[bp]],
                                                 start=st, stop=sp)
                            for bp in range(NBP):
                                nc.tensor.matmul(S1s[bp][:], Wb(m * 2 + kb),
                                                 Xb0f[:, isls[bp]],
                                                 start=st, stop=sp)
                            for bp in range(NBP):
                                nc.tensor.matmul(U4s[bp][:], W(4 + m * 2 + kb),
                                                 X1rf[:, isls[bp]],
                                                 start=st, stop=False)
                        # column-neighbor sums via 4I with +-1-shifted rhs:
                        # U4 = W_R2@X1 + 4I@X1[+1] + 4I@X1[-1], all in PSUM
                        for bp in range(NBP):
                            lo = m * (BCH * P) + bp * (2 * P)
                            hi = lo + 2 * P
                            # right neighbor (col+1)
                            nc.tensor.matmul(U4s[bp][:], W(12),
                                             X1p[:, lo + 2:hi + 2],
                                             start=False, stop=False)
                            nc.tensor.matmul(U4s[bp][:], W(12),
                                             X1p[:, lo:hi],
                                             start=False, stop=True)
                        for bp in range(NBP):
                            lo = m * (BCH * P) + bp * (2 * P)
                            sl = slice(lo, lo + 2 * P)
                            nc.scalar.copy(out=scp[:, sl], in_=S1s[bp][:])
                            nc.scalar.copy(out=rcp[:, sl], in_=R1s[bp][:])
                            # tm = (X0 + 1.5) * U4  (PSUM source)
                            stt(tmf[:, sl], X0f[:, sl], 1.5, U4s[bp][:],
                                ADD, MUL)

                    # gradient-product terms, both in the shifted layout,
                    # summed into PSUM via identity-matmul accumulation
                    nc.vector.tensor_mul(t2b[:, 0:FCH - 1], scp[:, 1:FCH],
                                         rcp[:, 1:FCH])
                    for s in range(FCH // (2 * P)):
                        sl = slice(s * 2 * P, (s + 1) * 2 * P)
                        ACC = pp.tile([128, 2 * P], f32, tag="acc",
                                      name=f"acc_{c}_{s}")
                        nc.tensor.matmul(ACC[:], ibf[:], t2b[:, sl],
                                         start=True, stop=False)
                        nc.tensor.matmul(ACC[:], ibf[:], t3b[:, sl],
                                         start=False, stop=True)
                        n = 2 * P if (s + 1) * 2 * P < FCH else 2 * P - 1
                        lo = s * 2 * P + 1
                        nc.vector.tensor_add(tmf[:, lo:lo + n],
                                             tmf[:, lo:lo + n], ACC[:, 0:n])

                    tmr = tm.rearrange("p k b j -> p (k b) j")
                    for m in range(2):
                        for bp in range(BCH // 2):
                            resv = tmr[:, m * BCH + bp * 2:
                                       m * BCH + bp * 2 + 2, 1:P - 1]
                            nc.scalar.activation(resv, resv, COPY,
                                                 bias=-1.0, scale=-GAMMA)

                    # border columns from the edge pipeline
                    for e, j in ((0, 0), (1, P - 1)):
                        esrc = (rese.rearrange("p (k b) e -> p k b e", k=2)
                                [:, :, b0c:b0c + BCH, e:e + 1])
                        nc.scalar.copy(out=tm[:, :, :, j:j + 1], in_=esrc)

                    nc.sync.dma_start(out=yout[:, :, b0c:b0c + BCH, 0:P],
                                      in_=tm[:])

    nc.compile()
    return nc


def _get_program():
    if "nc" not in _cache:
        _cache["nc"] = _build_program()
        _cache["wts"] = _weights()
        import ml_dtypes
        _cache["ident"] = np.eye(128, dtype=ml_dtypes.bfloat16)
        D1 = np.zeros((P, P))
        for i in range(1, P - 1):
            D1[i, i - 1] = -1.0
            D1[i, i + 1] = 1.0
        D1[0, 0:3] = [-3.0, 4.0, -1.0]
        D1[P - 1, P - 3:P] = [1.0, -4.0, 3.0]
        wtbd = np.zeros((128, 4, 128), dtype=ml_dtypes.bfloat16)
        for m in range(2):
            for kb in range(2):
                wtbd[:, m * 2 + kb, :] = D1[m * 128:(m + 1) * 128,
                                            kb * 128:(kb + 1) * 128].T
        _cache["wtbd"] = wtbd
    return _cache["nc"], _cache["wts"], _cache["ident"]


def _shard_inputs(x0_pred):
    import ml_dtypes
    x = np.ascontiguousarray(np.asarray(x0_pred, dtype=np.float32))
    _, wts, ident = _get_program()
    wtbd = _cache["wtbd"]
    in_maps = []
    for i in range(NCORES):
        shard = x[i * BPC:(i + 1) * BPC]                      # [16,2,256,256]
        arr = shard.reshape(BPC, 2, 2, 128, P).transpose(3, 1, 2, 0, 4)
        arr = np.ascontiguousarray(arr)
        cols = [0, 1, 2, 3, P - 4, P - 3, P - 2, P - 1]
        xe = np.ascontiguousarray(arr[:, :, :, :, cols])
        xbi = arr.astype(ml_dtypes.bfloat16)
        in_maps.append({"xin": arr, "xe": xe, "xb": xbi, "wts": wts,
                        "ident": ident, "wtbd": wtbd})
    return in_maps


def _unshard(results):
    outs = []
    for i in range(NCORES):
        y = results[i]["yout"]                                # [128,2,16,256]
        outs.append(y.transpose(2, 1, 0, 3).reshape(BPC, 1, P, P))
    return np.ascontiguousarray(np.concatenate(outs, axis=0))


def _run(x0_pred, trace=False, tmpdir=None):
    from concourse.bass_utils import run_bass_kernel_spmd
    nc = _get_program()[0]
    in_maps = _shard_inputs(x0_pred)
    res = run_bass_kernel_spmd(nc, in_maps, list(range(NCORES)),
                               trace=trace, tmpdir=tmpdir)
    return _unshard(res.results), res


def kernel(x0_pred):
    out, _ = _run(x0_pred, trace=False)
    return out


# revision 21
# speedup vs baseline: 1.0160x; 1.0160x over previous
"""Trainium2 Bass kernel for nn_DarcyResidual (P=256, B=128, 8 NeuronCores).

Math (reference):
    a = (x0 + 1.5) / 0.2,  p = (x1 + 0.9) / 115
    residual = -a*(p_d00 + p_d11) - a_d0*p_d0 - a_d1*p_d1 - 1
2nd-order central differences inside, 2nd-order one-sided at borders,
h = 1/256 on both axes.

Folded form computed here (G = 5/(460 h^2)):
    residual = -G * [ (X0 + 1.5)*U4 + S1*R1 + C1a*C1p ] - 1
      U4  = 4*(rowD2raw(X1) + colD2raw(X1))   (raw h^2-scaled 2nd diffs)
      R1  = rowD1raw(X1), S1 = rowD1raw(X0)   (raw 2h-scaled 1st diffs)
      C1p = colD1raw(X1), C1a = colD1raw(X0)

Layout per core (16 images): SBUF [partition = row-within-128-block,
free = (row-block k:2, image b, col j:256)].  Row (d0) derivatives are
TensorE matmuls with banded stencil matrices in fp32r (boundary rows are
rows of the matrices; the -2I of the column stencil is folded in as
W_R2 = 4*(D2 - 2I)).  Column (d1) stencils are shifted-AP DVE ops.  The
two small gradient-product terms are written as bf16 and summed in PSUM
via identity-matmul accumulation.  Border columns j=0,255 get their own
small one-sided pipeline.  ScalarE does PSUM evacuation + final affine.
"""

import numpy as np

P = 256
B = 128
NCORES = 8
BPC = B // NCORES          # images per core = 16
CHUNKS = 8
BCH = BPC // CHUNKS        # images per chunk = 2
FCH = 2 * BCH * P          # chunk free size = 2048
GAMMA = 5.0 * 65536.0 / 460.0

_cache = {}


def _weights():
    """[128, 12, 128] stacked lhsT blocks for the row-stencil matmuls."""
    D1 = np.zeros((P, P), dtype=np.float64)
    for i in range(1, P - 1):
        D1[i, i - 1] = -1.0
        D1[i, i + 1] = 1.0
    D1[0, 0:3] = [-3.0, 4.0, -1.0]
    D1[P - 1, P - 3:P] = [1.0, -4.0, 3.0]

    D2 = np.zeros((P, P), dtype=np.float64)
    for i in range(1, P - 1):
        D2[i, i - 1] = 1.0
        D2[i, i] = -2.0
        D2[i, i + 1] = 1.0
    D2[0, 0:4] = [2.0, -5.0, 4.0, -1.0]
    D2[P - 1, P - 4:P] = [-1.0, 4.0, -5.0, 2.0]

    WR2 = 4.0 * (D2 - 2.0 * np.eye(P))   # main (col -2I folded here)
    WR2E = 4.0 * D2                      # edge (col stencil complete)

    wts = np.zeros((128, 13, 128), dtype=np.float32)
    wts[:, 12, :] = 4.0 * np.eye(128)
    for m in range(2):
        for kb in range(2):
            i = m * 2 + kb
            blk = lambda W: W[m * 128:(m + 1) * 128, kb * 128:(kb + 1) * 128].T
            wts[:, i, :] = blk(D1)
            wts[:, 4 + i, :] = blk(WR2)
            wts[:, 8 + i, :] = blk(WR2E)
    return wts


def _build_program():
    from concourse import bacc
    import concourse.mybir as mybir
    from concourse.tile import TileContext
    import ml_dtypes

    f32 = mybir.dt.float32
    f32r = mybir.dt.float32r
    bf16 = mybir.dt.bfloat16
    ADD = mybir.AluOpType.add
    SUB = mybir.AluOpType.subtract
    MUL = mybir.AluOpType.mult
    COPY = mybir.ActivationFunctionType.Copy

    nc = bacc.Bacc("TRN2", target_bir_lowering=False, debug=False,
                   num_devices=NCORES)
    xin = nc.dram_tensor("xin", [128, 2, 2, BPC, P], f32r, kind="ExternalInput")
    xe = nc.dram_tensor("xe", [128, 2, 2, BPC, 8], f32r, kind="ExternalInput")
    wts = nc.dram_tensor("wts", [128, 13, 128], f32r, kind="ExternalInput")
    xb = nc.dram_tensor("xb", [128, 2, 2, BPC, P], bf16, kind="ExternalInput")
    ident = nc.dram_tensor("ident", [128, 128], bf16, kind="ExternalInput")
    wtbd = nc.dram_tensor("wtbd", [128, 4, 128], bf16, kind="ExternalInput")
    yout = nc.dram_tensor("yout", [128, 2, BPC, P], f32, kind="ExternalOutput")

    with TileContext(nc) as tc:
        with (
            tc.tile_pool(name="const", bufs=1) as cpool,
            tc.tile_pool(name="edge", bufs=1) as epool,
            tc.tile_pool(name="work", bufs=2) as pool,
            tc.tile_pool(name="psum", bufs=2, space="PSUM") as pp,
        ):
            wt = cpool.tile([128, 13, 128], f32r)
            nc.sync.dma_start(out=wt[:], in_=wts[:])
            ibf = cpool.tile([128, 128], bf16)
            nc.sync.dma_start(out=ibf[:], in_=ident[:])
            wtb = cpool.tile([128, 4, 128], bf16)
            nc.sync.dma_start(out=wtb[:], in_=wtbd[:])

            def Wb(i):
                return wtb[:, i, :]

            def W(i):
                return wt[:, i, :]

            stt = nc.vector.scalar_tensor_tensor

            # ------------- edge pipeline (output cols j=0 and j=255) -------
            # (emitted after chunk 0's body so chunk-0 stencils start first)
            X0e = epool.tile([128, 2, BPC, 8], f32r)
            X1e = epool.tile([128, 2, BPC, 8], f32r)
            nc.sync.dma_start(out=X0e[:], in_=xe[:, 0])
            nc.sync.dma_start(out=X1e[:], in_=xe[:, 1])

            X0ef = X0e.rearrange("p k b c -> p (k b c)")
            X1ef = X1e.rearrange("p k b c -> p (k b c)")
            # fp32 views, [128, 32, 8]
            E1 = X1e.bitcast(f32).rearrange("p k b c -> p (k b) c")
            E0 = X0e.bitcast(f32).rearrange("p k b c -> p (k b) c")

            def et(name, d=2):
                return epool.tile([128, 2 * BPC, d], f32, name=name, tag=name)

            if True:
                R2e = pp.tile([128, 2, BPC, 8], f32, tag="r2")
                R1e = pp.tile([128, 2, BPC, 8], f32, tag="r1")
                S1e = pp.tile([128, 2, BPC, 8], f32, tag="s1")
                R2ef = R2e.rearrange("p k b c -> p (k b c)")
                R1ef = R1e.rearrange("p k b c -> p (k b c)")
                S1ef = S1e.rearrange("p k b c -> p (k b c)")
                for m in range(2):
                    osl = slice(m * 128, (m + 1) * 128)
                    for kb in range(2):
                        isl = slice(kb * 128, (kb + 1) * 128)
                        st, sp = kb == 0, kb == 1
                        nc.tensor.matmul(R1ef[:, osl], W(m * 2 + kb),
                                         X1ef[:, isl], start=st, stop=sp)
                        nc.tensor.matmul(S1ef[:, osl], W(m * 2 + kb),
                                         X0ef[:, isl], start=st, stop=sp)
                        nc.tensor.matmul(R2ef[:, osl], W(8 + m * 2 + kb),
                                         X1ef[:, isl], start=st, stop=sp)

                # paired forward/mirrored diffs: half 0 = j=0 side (fwd),
                # half 1 = j=255 side (also forward-oriented: f7-f6 etc.)
                a1, b1, c1 = et("a1"), et("b1"), et("c1")
                a0, b0 = et("a0"), et("b0")
                nc.vector.tensor_sub(a1[:], E1[:, :, 1:8:6], E1[:, :, 0:7:6])
                nc.vector.tensor_sub(b1[:], E1[:, :, 2:7:4], E1[:, :, 1:6:4])
                nc.vector.tensor_sub(c1[:], E1[:, :, 3:6:2], E1[:, :, 2:5:2])
                nc.vector.tensor_sub(a0[:], E0[:, :, 1:8:6], E0[:, :, 0:7:6])
                nc.vector.tensor_sub(b0[:], E0[:, :, 2:7:4], E0[:, :, 1:6:4])

                # one-sided raw stencils (Z sign flips on the mirror half)
                q, Z = et("q"), et("Z")
                C1pe, C1ae = et("C1pe"), et("C1ae")
                stt(q[:], b1[:], 3.0, c1[:], MUL, SUB)      # 3b - c
                stt(Z[:], a1[:], -2.0, q[:], MUL, ADD)      # -2a + 3b - c
                stt(C1pe[:], a1[:], 3.0, b1[:], MUL, SUB)   # 3a - b
                stt(C1ae[:], a0[:], 3.0, b0[:], MUL, SUB)

                RP2 = R2e.rearrange("p k b c -> p (k b) c")
                RP1 = R1e.rearrange("p k b c -> p (k b) c")
                U4e, tme, t2e = et("U4e"), et("tme"), et("t2e")
                stt(U4e[:, :, 0:1], Z[:, :, 0:1], 4.0, RP2[:, :, 0:1], MUL, ADD)
                stt(U4e[:, :, 1:2], Z[:, :, 1:2], -4.0, RP2[:, :, 7:8], MUL, ADD)

                Scpe = epool.tile([128, 2, BPC, 8], f32)
                nc.scalar.copy(out=Scpe.rearrange("p k b c -> p (k b c)"),
                               in_=S1ef[:])
                SP = Scpe.rearrange("p k b c -> p (k b) c")

                stt(tme[:], E0[:, :, 0:8:7], 1.5, U4e[:], ADD, MUL)
                nc.vector.tensor_mul(t2e[:], SP[:, :, 0:8:7], RP1[:, :, 0:8:7])
                nc.vector.tensor_add(tme[:], tme[:], t2e[:])
                nc.vector.tensor_mul(C1ae[:], C1ae[:], C1pe[:])  # t3e in-place
                nc.vector.tensor_add(tme[:], tme[:], C1ae[:])
                rese = epool.tile([128, 2 * BPC, 2], f32)
                nc.scalar.activation(rese[:], tme[:], COPY,
                                     bias=-1.0, scale=-GAMMA)

            # ------------- main pipeline, 4 chunks of 4 images -------------
            if True:
                for c in range(CHUNKS):
                    b0c = c * BCH
                    XbC = pool.tile([128, 2, 2, BCH, P], bf16, tag="xbc",
                                    bufs=3)
                    nc.sync.dma_start(out=XbC[:], in_=xb[:, :, :, b0c:b0c + BCH, :])
                    X0c = pool.tile([128, 2, BCH, P], f32r, tag="x0", bufs=3)
                    # X1 padded by one col each side so the +-1-shifted
                    # identity-matmul rhs views stay in bounds at full N
                    X1p = pool.tile([128, FCH + 2], f32r, tag="x1", bufs=3)
                    nc.sync.dma_start(
                        out=X1p[:, 1:FCH + 1].rearrange(
                            "p (k b j) -> p k b j", k=2, b=BCH),
                        in_=xin[:, 1, :, b0c:b0c + BCH, :])
                    nc.sync.dma_start(out=X0c[:], in_=xin[:, 0, :, b0c:b0c + BCH, :])
                    XbCf = XbC.rearrange("p c k b j -> p (c k b j)")
                    Xb0f = XbCf[:, 0:FCH]
                    Xb1f = XbCf[:, FCH:2 * FCH]
                    X0rf = X0c.rearrange("p k b j -> p (k b j)")
                    X1rf = X1p[:, 1:FCH + 1]
                    X0f = X0c.bitcast(f32).rearrange("p k b j -> p (k b j)")
                    X1pf = X1p.bitcast(f32)
                    X1f = X1pf[:, 1:FCH + 1]
                    # "sh" tiles hold values for column index t+1 at slot t,
                    # keeping every bf16 operand 4-byte aligned (2x mode).
                    C1p = pool.tile([128, FCH], bf16, tag="c1p", bufs=3)
                    C1a = pool.tile([128, FCH], bf16, tag="c1a", bufs=3)
                    scp = pool.tile([128, FCH], f32, tag="scp", bufs=3)
                    rcp = pool.tile([128, FCH], f32, tag="rcp", bufs=3)
                    t2b = pool.tile([128, FCH], f32, tag="t2b", bufs=3)
                    t3b = pool.tile([128, FCH], bf16, tag="t3b", bufs=3)
                    tm = pool.tile([128, 2, BCH, P], f32, tag="tm", bufs=3)
                    tmf = tm.rearrange("p k b j -> p (k b j)")

                    # column stencils (interior; border cols handled above)
                    nc.vector.tensor_sub(C1p[:, 0:FCH - 2], Xb1f[:, 2:FCH],
                                         Xb1f[:, 0:FCH - 2])
                    nc.vector.tensor_sub(C1a[:, 0:FCH - 2], Xb0f[:, 2:FCH],
                                         Xb0f[:, 0:FCH - 2])
                    nc.vector.tensor_mul(t3b[:], C1a[:], C1p[:])

                    for m in range(2):
                        NBP = BCH // 2
                        R1s = [pp.tile([128, 2 * P], f32, name=f"r1_{c}_{m}_{bp}",
                                       tag="r1") for bp in range(NBP)]
                        S1s = [pp.tile([128, 2 * P], f32, name=f"s1_{c}_{m}_{bp}",
                                       tag="s1") for bp in range(NBP)]
                        U4s = [pp.tile([128, 2 * P], f32, name=f"r2_{c}_{m}_{bp}",
                                       tag="r2") for bp in range(NBP)]
                        for kb in range(2):
                            st, sp = kb == 0, kb == 1
                            isls = [slice(kb * (BCH * P) + bp * (2 * P),
                                          kb * (BCH * P) + (bp + 1) * (2 * P))
                                    for bp in range(NBP)]
                            for bp in range(NBP):
                                nc.tensor.matmul(R1s[bp][:], Wb(m * 2 + kb),
                                                 Xb1f[:, isls[bp]],
                                                 start=st, stop=sp)
                            for bp in range(NBP):
                                nc.tensor.matmul(S1s[bp][:], Wb(m * 2 + kb),
                                                 Xb0f[:, isls[bp]],
                                                 start=st, stop=sp)
                            for bp in range(NBP):
                                nc.tensor.matmul(U4s[bp][:], W(4 + m * 2 + kb),
                                                 X1rf[:, isls[bp]],
                                                 start=st, stop=False)
                        # column-neighbor sums via 4I with +-1-shifted rhs:
                        # U4 = W_R2@X1 + 4I@X1[+1] + 4I@X1[-1], all in PSUM
                        for bp in range(NBP):
                            lo = m * (BCH * P) + bp * (2 * P)
                            hi = lo + 2 * P
                            # right neighbor (col+1)
                            nc.tensor.matmul(U4s[bp][:], W(12),
                                             X1p[:, lo + 2:hi + 2],
                                             start=False, stop=False)
                            nc.tensor.matmul(U4s[bp][:], W(12),
                                             X1p[:, lo:hi],
                                             start=False, stop=True)
                        for bp in range(NBP):
                            lo = m * (BCH * P) + bp * (2 * P)
                            sl = slice(lo, lo + 2 * P)
                            nc.scalar.copy(out=scp[:, sl], in_=S1s[bp][:])
                            nc.scalar.copy(out=rcp[:, sl], in_=R1s[bp][:])
                            # tm = (X0 + 1.5) * U4  (PSUM source)
                            stt(tmf[:, sl], X0f[:, sl], 1.5, U4s[bp][:],
                                ADD, MUL)

                    # gradient-product terms added on DVE:
                    # t2b (aligned, fp32), t3b (shifted by one, bf16)
                    nc.vector.tensor_mul(t2b[:], scp[:], rcp[:])
                    nc.vector.tensor_add(tmf[:], tmf[:], t2b[:])
                    nc.vector.tensor_add(tmf[:, 1:FCH], tmf[:, 1:FCH],
                                         t3b[:, 0:FCH - 1])

                    tmr = tm.rearrange("p k b j -> p (k b) j")
                    for m in range(2):
                        for bp in range(BCH // 2):
                            resv = tmr[:, m * BCH + bp * 2:
                                       m * BCH + bp * 2 + 2, 1:P - 1]
                            nc.scalar.activation(resv, resv, COPY,
                                                 bias=-1.0, scale=-GAMMA)

                    # border columns from the edge pipeline
                    for e, j in ((0, 0), (1, P - 1)):
                        esrc = (rese.rearrange("p (k b) e -> p k b e", k=2)
                                [:, :, b0c:b0c + BCH, e:e + 1])
                        nc.scalar.copy(out=tm[:, :, :, j:j + 1], in_=esrc)

                    nc.sync.dma_start(out=yout[:, :, b0c:b0c + BCH, 0:P],
                                      in_=tm[:])

    nc.compile()
    return nc


def _get_program():
    if "nc" not in _cache:
        _cache["nc"] = _build_program()
        _cache["wts"] = _weights()
        import ml_dtypes
        _cache["ident"] = np.eye(128, dtype=ml_dtypes.bfloat16)
        D1 = np.zeros((P, P))
        for i in range(1, P - 1):
            D1[i, i - 1] = -1.0
            D1[i, i + 1] = 1.0
        D1[0, 0:3] = [-3.0, 4.0, -1.0]
        D1[P - 1, P - 3:P] = [1.0, -4.0, 3.0]
        wtbd = np.zeros((128, 4, 128), dtype=ml_dtypes.bfloat16)
        for m in range(2):
            for kb in range(2):
                wtbd[:, m * 2 + kb, :] = D1[m * 128:(m + 1) * 128,
                                            kb * 128:(kb + 1) * 128].T
        _cache["wtbd"] = wtbd
    return _cache["nc"], _cache["wts"], _cache["ident"]


def _shard_inputs(x0_pred):
    import ml_dtypes
    x = np.ascontiguousarray(np.asarray(x0_pred, dtype=np.float32))
    _, wts, ident = _get_program()
    wtbd = _cache["wtbd"]
    in_maps = []
    for i in range(NCORES):
        shard = x[i * BPC:(i + 1) * BPC]                      # [16,2,256,256]
        arr = shard.reshape(BPC, 2, 2, 128, P).transpose(3, 1, 2, 0, 4)
        arr = np.ascontiguousarray(arr)
        cols = [0, 1, 2, 3, P - 4, P - 3, P - 2, P - 1]
        xe = np.ascontiguousarray(arr[:, :, :, :, cols])
        xbi = arr.astype(ml_dtypes.bfloat16)
        in_maps.append({"xin": arr, "xe": xe, "xb": xbi, "wts": wts,
                        "ident": ident, "wtbd": wtbd})
    return in_maps


def _unshard(results):
    outs = []
    for i in range(NCORES):
        y = results[i]["yout"]                                # [128,2,16,256]
        outs.append(y.transpose(2, 1, 0, 3).reshape(BPC, 1, P, P))
    return np.ascontiguousarray(np.concatenate(outs, axis=0))


def _run(x0_pred, trace=False, tmpdir=None):
    from concourse.bass_utils import run_bass_kernel_spmd
    nc = _get_program()[0]
    in_maps = _shard_inputs(x0_pred)
    res = run_bass_kernel_spmd(nc, in_maps, list(range(NCORES)),
                               trace=trace, tmpdir=tmpdir)
    return _unshard(res.results), res


def kernel(x0_pred):
    out, _ = _run(x0_pred, trace=False)
    return out


# revision 22
# speedup vs baseline: 1.0169x; 1.0009x over previous
"""Trainium2 Bass kernel for nn_DarcyResidual (P=256, B=128, 8 NeuronCores).

Math (reference):
    a = (x0 + 1.5) / 0.2,  p = (x1 + 0.9) / 115
    residual = -a*(p_d00 + p_d11) - a_d0*p_d0 - a_d1*p_d1 - 1
2nd-order central differences inside, 2nd-order one-sided at borders,
h = 1/256 on both axes.

Folded form computed here (G = 5/(460 h^2)):
    residual = -G * [ (X0 + 1.5)*U4 + S1*R1 + C1a*C1p ] - 1
      U4  = 4*(rowD2raw(X1) + colD2raw(X1))   (raw h^2-scaled 2nd diffs)
      R1  = rowD1raw(X1), S1 = rowD1raw(X0)   (raw 2h-scaled 1st diffs)
      C1p = colD1raw(X1), C1a = colD1raw(X0)

Layout per core (16 images): SBUF [partition = row-within-128-block,
free = (row-block k:2, image b, col j:256)].  Row (d0) derivatives are
TensorE matmuls with banded stencil matrices in fp32r (boundary rows are
rows of the matrices; the -2I of the column stencil is folded in as
W_R2 = 4*(D2 - 2I)).  Column (d1) stencils are shifted-AP DVE ops.  The
two small gradient-product terms are written as bf16 and summed in PSUM
via identity-matmul accumulation.  Border columns j=0,255 get their own
small one-sided pipeline.  ScalarE does PSUM evacuation + final affine.
"""

import numpy as np

P = 256
B = 128
NCORES = 8
BPC = B // NCORES          # images per core = 16
CHUNKS = 8
BCH = BPC // CHUNKS        # images per chunk = 2
FCH = 2 * BCH * P          # chunk free size = 2048
GAMMA = 5.0 * 65536.0 / 460.0

_cache = {}


def _weights():
    """[128, 12, 128] stacked lhsT blocks for the row-stencil matmuls."""
    D1 = np.zeros((P, P), dtype=np.float64)
    for i in range(1, P - 1):
        D1[i, i - 1] = -1.0
        D1[i, i + 1] = 1.0
    D1[0, 0:3] = [-3.0, 4.0, -1.0]
    D1[P - 1, P - 3:P] = [1.0, -4.0, 3.0]

    D2 = np.zeros((P, P), dtype=np.float64)
    for i in range(1, P - 1):
        D2[i, i - 1] = 1.0
        D2[i, i] = -2.0
        D2[i, i + 1] = 1.0
    D2[0, 0:4] = [2.0, -5.0, 4.0, -1.0]
    D2[P - 1, P - 4:P] = [-1.0, 4.0, -5.0, 2.0]

    WR2 = 4.0 * (D2 - 2.0 * np.eye(P))   # main (col -2I folded here)
    WR2E = 4.0 * D2                      # edge (col stencil complete)

    wts = np.zeros((128, 13, 128), dtype=np.float32)
    wts[:, 12, :] = 4.0 * np.eye(128)
    for m in range(2):
        for kb in range(2):
            i = m * 2 + kb
            blk = lambda W: W[m * 128:(m + 1) * 128, kb * 128:(kb + 1) * 128].T
            wts[:, i, :] = blk(D1)
            wts[:, 4 + i, :] = blk(WR2)
            wts[:, 8 + i, :] = blk(WR2E)
    return wts


def _build_program():
    from concourse import bacc
    import concourse.mybir as mybir
    from concourse.tile import TileContext
    import ml_dtypes

    f32 = mybir.dt.float32
    f32r = mybir.dt.float32r
    bf16 = mybir.dt.bfloat16
    ADD = mybir.AluOpType.add
    SUB = mybir.AluOpType.subtract
    MUL = mybir.AluOpType.mult
    COPY = mybir.ActivationFunctionType.Copy

    nc = bacc.Bacc("TRN2", target_bir_lowering=False, debug=False,
                   num_devices=NCORES)
    xin = nc.dram_tensor("xin", [128, 2, 2, BPC, P], f32r, kind="ExternalInput")
    xe = nc.dram_tensor("xe", [128, 2, 2, BPC, 8], f32r, kind="ExternalInput")
    wts = nc.dram_tensor("wts", [128, 13, 128], f32r, kind="ExternalInput")
    xb = nc.dram_tensor("xb", [128, 2, 2, BPC, P], bf16, kind="ExternalInput")
    ident = nc.dram_tensor("ident", [128, 128], bf16, kind="ExternalInput")
    wtbd = nc.dram_tensor("wtbd", [128, 4, 128], bf16, kind="ExternalInput")
    yout = nc.dram_tensor("yout", [128, 2, BPC, P], f32, kind="ExternalOutput")

    with TileContext(nc) as tc:
        with (
            tc.tile_pool(name="const", bufs=1) as cpool,
            tc.tile_pool(name="edge", bufs=1) as epool,
            tc.tile_pool(name="work", bufs=2) as pool,
            tc.tile_pool(name="psum", bufs=2, space="PSUM") as pp,
        ):
            wt = cpool.tile([128, 13, 128], f32r)
            nc.sync.dma_start(out=wt[:], in_=wts[:])
            ibf = cpool.tile([128, 128], bf16)
            nc.sync.dma_start(out=ibf[:], in_=ident[:])
            wtb = cpool.tile([128, 4, 128], bf16)
            nc.sync.dma_start(out=wtb[:], in_=wtbd[:])

            def Wb(i):
                return wtb[:, i, :]

            def W(i):
                return wt[:, i, :]

            stt = nc.vector.scalar_tensor_tensor

            # ------------- edge pipeline (output cols j=0 and j=255) -------
            # (emitted after chunk 0's body so chunk-0 stencils start first)
            X0e = epool.tile([128, 2, BPC, 8], f32r)
            X1e = epool.tile([128, 2, BPC, 8], f32r)
            nc.sync.dma_start(out=X0e[:], in_=xe[:, 0])
            nc.sync.dma_start(out=X1e[:], in_=xe[:, 1])

            X0ef = X0e.rearrange("p k b c -> p (k b c)")
            X1ef = X1e.rearrange("p k b c -> p (k b c)")
            # fp32 views, [128, 32, 8]
            E1 = X1e.bitcast(f32).rearrange("p k b c -> p (k b) c")
            E0 = X0e.bitcast(f32).rearrange("p k b c -> p (k b) c")

            def et(name, d=2):
                return epool.tile([128, 2 * BPC, d], f32, name=name, tag=name)

            if True:
                R2e = pp.tile([128, 2, BPC, 8], f32, tag="r2")
                R1e = pp.tile([128, 2, BPC, 8], f32, tag="r1")
                S1e = pp.tile([128, 2, BPC, 8], f32, tag="s1")
                R2ef = R2e.rearrange("p k b c -> p (k b c)")
                R1ef = R1e.rearrange("p k b c -> p (k b c)")
                S1ef = S1e.rearrange("p k b c -> p (k b c)")
                for m in range(2):
                    osl = slice(m * 128, (m + 1) * 128)
                    for kb in range(2):
                        isl = slice(kb * 128, (kb + 1) * 128)
                        st, sp = kb == 0, kb == 1
                        nc.tensor.matmul(R1ef[:, osl], W(m * 2 + kb),
                                         X1ef[:, isl], start=st, stop=sp)
                        nc.tensor.matmul(S1ef[:, osl], W(m * 2 + kb),
                                         X0ef[:, isl], start=st, stop=sp)
                        nc.tensor.matmul(R2ef[:, osl], W(8 + m * 2 + kb),
                                         X1ef[:, isl], start=st, stop=sp)

                # paired forward/mirrored diffs: half 0 = j=0 side (fwd),
                # half 1 = j=255 side (also forward-oriented: f7-f6 etc.)
                a1, b1, c1 = et("a1"), et("b1"), et("c1")
                a0, b0 = et("a0"), et("b0")
                nc.vector.tensor_sub(a1[:], E1[:, :, 1:8:6], E1[:, :, 0:7:6])
                nc.vector.tensor_sub(b1[:], E1[:, :, 2:7:4], E1[:, :, 1:6:4])
                nc.vector.tensor_sub(c1[:], E1[:, :, 3:6:2], E1[:, :, 2:5:2])
                nc.vector.tensor_sub(a0[:], E0[:, :, 1:8:6], E0[:, :, 0:7:6])
                nc.vector.tensor_sub(b0[:], E0[:, :, 2:7:4], E0[:, :, 1:6:4])

                # one-sided raw stencils (Z sign flips on the mirror half)
                q, Z = et("q"), et("Z")
                C1pe, C1ae = et("C1pe"), et("C1ae")
                stt(q[:], b1[:], 3.0, c1[:], MUL, SUB)      # 3b - c
                stt(Z[:], a1[:], -2.0, q[:], MUL, ADD)      # -2a + 3b - c
                stt(C1pe[:], a1[:], 3.0, b1[:], MUL, SUB)   # 3a - b
                stt(C1ae[:], a0[:], 3.0, b0[:], MUL, SUB)

                RP2 = R2e.rearrange("p k b c -> p (k b) c")
                RP1 = R1e.rearrange("p k b c -> p (k b) c")
                U4e, tme, t2e = et("U4e"), et("tme"), et("t2e")
                stt(U4e[:, :, 0:1], Z[:, :, 0:1], 4.0, RP2[:, :, 0:1], MUL, ADD)
                stt(U4e[:, :, 1:2], Z[:, :, 1:2], -4.0, RP2[:, :, 7:8], MUL, ADD)

                Scpe = epool.tile([128, 2, BPC, 8], f32)
                nc.scalar.copy(out=Scpe.rearrange("p k b c -> p (k b c)"),
                               in_=S1ef[:])
                SP = Scpe.rearrange("p k b c -> p (k b) c")

                stt(tme[:], E0[:, :, 0:8:7], 1.5, U4e[:], ADD, MUL)
                nc.vector.tensor_mul(t2e[:], SP[:, :, 0:8:7], RP1[:, :, 0:8:7])
                nc.vector.tensor_add(tme[:], tme[:], t2e[:])
                nc.vector.tensor_mul(C1ae[:], C1ae[:], C1pe[:])  # t3e in-place
                nc.vector.tensor_add(tme[:], tme[:], C1ae[:])
                rese = epool.tile([128, 2 * BPC, 2], f32)
                nc.scalar.activation(rese[:], tme[:], COPY,
                                     bias=-1.0, scale=-GAMMA)

            # ------------- main pipeline, 4 chunks of 4 images -------------
            if True:
                for c in range(CHUNKS):
                    b0c = c * BCH
                    XbC = pool.tile([128, 2, 2, BCH, P], bf16, tag="xbc",
                                    bufs=3)
                    nc.sync.dma_start(out=XbC[:], in_=xb[:, :, :, b0c:b0c + BCH, :])
                    X0c = pool.tile([128, 2, BCH, P], f32r, tag="x0", bufs=3)
                    # X1 padded by one col each side so the +-1-shifted
                    # identity-matmul rhs views stay in bounds at full N
                    X1p = pool.tile([128, FCH + 2], f32r, tag="x1", bufs=3)
                    nc.sync.dma_start(
                        out=X1p[:, 1:FCH + 1].rearrange(
                            "p (k b j) -> p k b j", k=2, b=BCH),
                        in_=xin[:, 1, :, b0c:b0c + BCH, :])
                    nc.sync.dma_start(out=X0c[:], in_=xin[:, 0, :, b0c:b0c + BCH, :])
                    XbCf = XbC.rearrange("p c k b j -> p (c k b j)")
                    Xb0f = XbCf[:, 0:FCH]
                    Xb1f = XbCf[:, FCH:2 * FCH]
                    X0rf = X0c.rearrange("p k b j -> p (k b j)")
                    X1rf = X1p[:, 1:FCH + 1]
                    X0f = X0c.bitcast(f32).rearrange("p k b j -> p (k b j)")
                    X1pf = X1p.bitcast(f32)
                    X1f = X1pf[:, 1:FCH + 1]
                    # "sh" tiles hold values for column index t+1 at slot t,
                    # keeping every bf16 operand 4-byte aligned (2x mode).
                    C1p = pool.tile([128, FCH], bf16, tag="c1p", bufs=3)
                    C1a = pool.tile([128, FCH], bf16, tag="c1a", bufs=3)
                    scp = pool.tile([128, FCH], f32, tag="scp", bufs=3)
                    rcp = pool.tile([128, FCH], f32, tag="rcp", bufs=3)
                    t2b = pool.tile([128, FCH], f32, tag="t2b", bufs=3)
                    t3b = pool.tile([128, FCH], bf16, tag="t3b", bufs=3)
                    tm = pool.tile([128, 2, BCH, P], f32, tag="tm", bufs=3)
                    tmf = tm.rearrange("p k b j -> p (k b j)")

                    # column stencils (interior; border cols handled above)
                    nc.vector.tensor_sub(C1p[:, 0:FCH - 2], Xb1f[:, 2:FCH],
                                         Xb1f[:, 0:FCH - 2])
                    nc.vector.tensor_sub(C1a[:, 0:FCH - 2], Xb0f[:, 2:FCH],
                                         Xb0f[:, 0:FCH - 2])
                    nc.vector.tensor_mul(t3b[:], C1a[:], C1p[:])

                    for m in range(2):
                        NBP = BCH // 2
                        R1s = [pp.tile([128, 2 * P], f32, name=f"r1_{c}_{m}_{bp}",
                                       tag="r1") for bp in range(NBP)]
                        S1s = [pp.tile([128, 2 * P], f32, name=f"s1_{c}_{m}_{bp}",
                                       tag="s1") for bp in range(NBP)]
                        U4s = [pp.tile([128, 2 * P], f32, name=f"r2_{c}_{m}_{bp}",
                                       tag="r2") for bp in range(NBP)]
                        for kb in range(2):
                            st, sp = kb == 0, kb == 1
                            isls = [slice(kb * (BCH * P) + bp * (2 * P),
                                          kb * (BCH * P) + (bp + 1) * (2 * P))
                                    for bp in range(NBP)]
                            for bp in range(NBP):
                                nc.tensor.matmul(R1s[bp][:], Wb(m * 2 + kb),
                                                 Xb1f[:, isls[bp]],
                                                 start=st, stop=sp)
                            for bp in range(NBP):
                                nc.tensor.matmul(S1s[bp][:], Wb(m * 2 + kb),
                                                 Xb0f[:, isls[bp]],
                                                 start=st, stop=sp)
                            for bp in range(NBP):
                                nc.tensor.matmul(U4s[bp][:], W(4 + m * 2 + kb),
                                                 X1rf[:, isls[bp]],
                                                 start=st, stop=False)
                        # column-neighbor sums via 4I with +-1-shifted rhs:
                        # U4 = W_R2@X1 + 4I@X1[+1] + 4I@X1[-1], all in PSUM
                        for bp in range(NBP):
                            lo = m * (BCH * P) + bp * (2 * P)
                            hi = lo + 2 * P
                            # right neighbor (col+1)
                            nc.tensor.matmul(U4s[bp][:], W(12),
                                             X1p[:, lo + 2:hi + 2],
                                             start=False, stop=False)
                            nc.tensor.matmul(U4s[bp][:], W(12),
                                             X1p[:, lo:hi],
                                             start=False, stop=True)
                        for bp in range(NBP):
                            lo = m * (BCH * P) + bp * (2 * P)
                            sl = slice(lo, lo + 2 * P)
                            nc.scalar.copy(out=scp[:, sl], in_=S1s[bp][:])
                            nc.scalar.copy(out=rcp[:, sl], in_=R1s[bp][:])
                            # tm = (X0 + 1.5) * U4  (PSUM source)
                            stt(tmf[:, sl], X0f[:, sl], 1.5, U4s[bp][:],
                                ADD, MUL)

                    # gradient-product terms added on DVE:
                    # t2b (aligned, fp32), t3b (shifted by one, bf16)
                    nc.vector.tensor_mul(t2b[:], scp[:], rcp[:])
                    nc.vector.tensor_add(tmf[:], tmf[:], t2b[:])
                    nc.vector.tensor_add(tmf[:, 1:FCH], tmf[:, 1:FCH],
                                         t3b[:, 0:FCH - 1])

                    tmr = tm.rearrange("p k b j -> p (k b) j")
                    for m in range(2):
                        for bp in range(BCH // 2):
                            resv = tmr[:, m * BCH + bp * 2:
                                       m * BCH + bp * 2 + 2, 1:P - 1]
                            nc.scalar.activation(resv, resv, COPY,
                                                 bias=-1.0, scale=-GAMMA)

                    # border columns from the edge pipeline
                    for e, j in ((0, 0), (1, P - 1)):
                        esrc = (rese.rearrange("p (k b) e -> p k b e", k=2)
                                [:, :, b0c:b0c + BCH, e:e + 1])
                        nc.scalar.copy(out=tm[:, :, :, j:j + 1], in_=esrc)

                    nc.gpsimd.dma_start(out=yout[:, :, b0c:b0c + BCH, 0:P],
                                        in_=tm[:])

    nc.compile()
    return nc


def _get_program():
    if "nc" not in _cache:
        _cache["nc"] = _build_program()
        _cache["wts"] = _weights()
        import ml_dtypes
        _cache["ident"] = np.eye(128, dtype=ml_dtypes.bfloat16)
        D1 = np.zeros((P, P))
        for i in range(1, P - 1):
            D1[i, i - 1] = -1.0
            D1[i, i + 1] = 1.0
        D1[0, 0:3] = [-3.0, 4.0, -1.0]
        D1[P - 1, P - 3:P] = [1.0, -4.0, 3.0]
        wtbd = np.zeros((128, 4, 128), dtype=ml_dtypes.bfloat16)
        for m in range(2):
            for kb in range(2):
                wtbd[:, m * 2 + kb, :] = D1[m * 128:(m + 1) * 128,
                                            kb * 128:(kb + 1) * 128].T
        _cache["wtbd"] = wtbd
    return _cache["nc"], _cache["wts"], _cache["ident"]


def _shard_inputs(x0_pred):
    import ml_dtypes
    x = np.ascontiguousarray(np.asarray(x0_pred, dtype=np.float32))
    _, wts, ident = _get_program()
    wtbd = _cache["wtbd"]
    in_maps = []
    for i in range(NCORES):
        shard = x[i * BPC:(i + 1) * BPC]                      # [16,2,256,256]
        arr = shard.reshape(BPC, 2, 2, 128, P).transpose(3, 1, 2, 0, 4)
        arr = np.ascontiguousarray(arr)
        cols = [0, 1, 2, 3, P - 4, P - 3, P - 2, P - 1]
        xe = np.ascontiguousarray(arr[:, :, :, :, cols])
        xbi = arr.astype(ml_dtypes.bfloat16)
        in_maps.append({"xin": arr, "xe": xe, "xb": xbi, "wts": wts,
                        "ident": ident, "wtbd": wtbd})
    return in_maps


def _unshard(results):
    outs = []
    for i in range(NCORES):
        y = results[i]["yout"]                                # [128,2,16,256]
        outs.append(y.transpose(2, 1, 0, 3).reshape(BPC, 1, P, P))
    return np.ascontiguousarray(np.concatenate(outs, axis=0))


def _run(x0_pred, trace=False, tmpdir=None):
    from concourse.bass_utils import run_bass_kernel_spmd
    nc = _get_program()[0]
    in_maps = _shard_inputs(x0_pred)
    res = run_bass_kernel_spmd(nc, in_maps, list(range(NCORES)),
                               trace=trace, tmpdir=tmpdir)
    return _unshard(res.results), res


def kernel(x0_pred):
    out, _ = _run(x0_pred, trace=False)
    return out


# revision 24
# speedup vs baseline: 1.1430x; 1.1240x over previous
"""Trainium2 Bass kernel for nn_DarcyResidual (P=256, B=128, 8 NeuronCores).

Math (reference):
    a = (x0 + 1.5) / 0.2,  p = (x1 + 0.9) / 115
    residual = -a*(p_d00 + p_d11) - a_d0*p_d0 - a_d1*p_d1 - 1
2nd-order central differences inside, 2nd-order one-sided at borders,
h = 1/256 on both axes.

Folded form computed here (G = 5/(460 h^2)):
    residual = -G * [ (X0 + 1.5)*U4 + S1*R1 + C1a*C1p ] - 1
      U4  = 4*(rowD2raw(X1) + colD2raw(X1))   (raw h^2-scaled 2nd diffs)
      R1  = rowD1raw(X1), S1 = rowD1raw(X0)   (raw 2h-scaled 1st diffs)
      C1p = colD1raw(X1), C1a = colD1raw(X0)

Layout per core (16 images): SBUF [partition = row-within-128-block,
free = (row-block k:2, image b, col j:256)].  Row (d0) derivatives are
TensorE matmuls with banded stencil matrices in fp32r (boundary rows are
rows of the matrices; the -2I of the column stencil is folded in as
W_R2 = 4*(D2 - 2I)).  Column (d1) stencils are shifted-AP DVE ops.  The
two small gradient-product terms are written as bf16 and summed in PSUM
via identity-matmul accumulation.  Border columns j=0,255 get their own
small one-sided pipeline.  ScalarE does PSUM evacuation + final affine.
"""

import numpy as np

P = 256
B = 128
NCORES = 8
BPC = B // NCORES          # images per core = 16
CHUNKS = 8
BCH = BPC // CHUNKS        # images per chunk = 2
FCH = 2 * BCH * P          # chunk free size = 2048
GAMMA = 5.0 * 65536.0 / 460.0

_cache = {}


def _weights():
    """[128, 12, 128] stacked lhsT blocks for the row-stencil matmuls."""
    D1 = np.zeros((P, P), dtype=np.float64)
    for i in range(1, P - 1):
        D1[i, i - 1] = -1.0
        D1[i, i + 1] = 1.0
    D1[0, 0:3] = [-3.0, 4.0, -1.0]
    D1[P - 1, P - 3:P] = [1.0, -4.0, 3.0]

    D2 = np.zeros((P, P), dtype=np.float64)
    for i in range(1, P - 1):
        D2[i, i - 1] = 1.0
        D2[i, i] = -2.0
        D2[i, i + 1] = 1.0
    D2[0, 0:4] = [2.0, -5.0, 4.0, -1.0]
    D2[P - 1, P - 4:P] = [-1.0, 4.0, -5.0, 2.0]

    WR2 = 4.0 * (D2 - 2.0 * np.eye(P))   # main (col -2I folded here)
    WR2E = 4.0 * D2                      # edge (col stencil complete)

    wts = np.zeros((128, 13, 128), dtype=np.float32)
    wts[:, 12, :] = 4.0 * np.eye(128)
    for m in range(2):
        for kb in range(2):
            i = m * 2 + kb
            blk = lambda W: W[m * 128:(m + 1) * 128, kb * 128:(kb + 1) * 128].T
            wts[:, i, :] = blk(D1)
            wts[:, 4 + i, :] = blk(WR2)
            wts[:, 8 + i, :] = blk(WR2E)
    return wts


def _build_program():
    from concourse import bacc
    import concourse.mybir as mybir
    from concourse.tile import TileContext
    import ml_dtypes

    f32 = mybir.dt.float32
    f32r = mybir.dt.float32r
    bf16 = mybir.dt.bfloat16
    ADD = mybir.AluOpType.add
    SUB = mybir.AluOpType.subtract
    MUL = mybir.AluOpType.mult
    COPY = mybir.ActivationFunctionType.Copy

    nc = bacc.Bacc("TRN2", target_bir_lowering=False, debug=False,
                   num_devices=NCORES)
    xin = nc.dram_tensor("xin", [128, 2, 2, BPC, P], f32r, kind="ExternalInput")
    xe = nc.dram_tensor("xe", [128, 2, 2, BPC, 8], f32r, kind="ExternalInput")
    wts = nc.dram_tensor("wts", [128, 13, 128], f32r, kind="ExternalInput")
    xb = nc.dram_tensor("xb", [128, 2, 2, BPC, P], bf16, kind="ExternalInput")
    ident = nc.dram_tensor("ident", [128, 128], bf16, kind="ExternalInput")
    wtbd = nc.dram_tensor("wtbd", [128, 4, 128], bf16, kind="ExternalInput")
    yout = nc.dram_tensor("yout", [128, 2, BPC, P], f32, kind="ExternalOutput")

    with TileContext(nc) as tc:
        with (
            tc.tile_pool(name="const", bufs=1) as cpool,
            tc.tile_pool(name="edge", bufs=1) as epool,
            tc.tile_pool(name="work", bufs=2) as pool,
            tc.tile_pool(name="psum", bufs=2, space="PSUM") as pp,
        ):
            wt = cpool.tile([128, 13, 128], f32r)
            nc.sync.dma_start(out=wt[:], in_=wts[:])
            ibf = cpool.tile([128, 128], bf16)
            nc.sync.dma_start(out=ibf[:], in_=ident[:])
            wtb = cpool.tile([128, 4, 128], bf16)
            nc.sync.dma_start(out=wtb[:], in_=wtbd[:])

            def Wb(i):
                return wtb[:, i, :]

            def W(i):
                return wt[:, i, :]

            stt = nc.vector.scalar_tensor_tensor

            # ------------- edge pipeline (output cols j=0 and j=255) -------
            # (emitted after chunk 0's body so chunk-0 stencils start first)
            X0e = epool.tile([128, 2, BPC, 8], f32r)
            X1e = epool.tile([128, 2, BPC, 8], f32r)
            nc.sync.dma_start(out=X0e[:], in_=xe[:, 0])
            nc.sync.dma_start(out=X1e[:], in_=xe[:, 1])

            X0ef = X0e.rearrange("p k b c -> p (k b c)")
            X1ef = X1e.rearrange("p k b c -> p (k b c)")
            # fp32 views, [128, 32, 8]
            E1 = X1e.bitcast(f32).rearrange("p k b c -> p (k b) c")
            E0 = X0e.bitcast(f32).rearrange("p k b c -> p (k b) c")

            def et(name, d=2):
                return epool.tile([128, 2 * BPC, d], f32, name=name, tag=name)

            if True:
                R2e = pp.tile([128, 2, BPC, 8], f32, tag="r2")
                R1e = pp.tile([128, 2, BPC, 8], f32, tag="r1")
                S1e = pp.tile([128, 2, BPC, 8], f32, tag="s1")
                R2ef = R2e.rearrange("p k b c -> p (k b c)")
                R1ef = R1e.rearrange("p k b c -> p (k b c)")
                S1ef = S1e.rearrange("p k b c -> p (k b c)")
                for m in range(2):
                    osl = slice(m * 128, (m + 1) * 128)
                    for kb in range(2):
                        isl = slice(kb * 128, (kb + 1) * 128)
                        st, sp = kb == 0, kb == 1
                        nc.tensor.matmul(R1ef[:, osl], W(m * 2 + kb),
                                         X1ef[:, isl], start=st, stop=sp)
                        nc.tensor.matmul(S1ef[:, osl], W(m * 2 + kb),
                                         X0ef[:, isl], start=st, stop=sp)
                        nc.tensor.matmul(R2ef[:, osl], W(8 + m * 2 + kb),
                                         X1ef[:, isl], start=st, stop=sp)

                # paired forward/mirrored diffs: half 0 = j=0 side (fwd),
                # half 1 = j=255 side (also forward-oriented: f7-f6 etc.)
                a1, b1, c1 = et("a1"), et("b1"), et("c1")
                a0, b0 = et("a0"), et("b0")
                nc.vector.tensor_sub(a1[:], E1[:, :, 1:8:6], E1[:, :, 0:7:6])
                nc.vector.tensor_sub(b1[:], E1[:, :, 2:7:4], E1[:, :, 1:6:4])
                nc.vector.tensor_sub(c1[:], E1[:, :, 3:6:2], E1[:, :, 2:5:2])
                nc.vector.tensor_sub(a0[:], E0[:, :, 1:8:6], E0[:, :, 0:7:6])
                nc.vector.tensor_sub(b0[:], E0[:, :, 2:7:4], E0[:, :, 1:6:4])

                # one-sided raw stencils (Z sign flips on the mirror half)
                q, Z = et("q"), et("Z")
                C1pe, C1ae = et("C1pe"), et("C1ae")
                stt(q[:], b1[:], 3.0, c1[:], MUL, SUB)      # 3b - c
                stt(Z[:], a1[:], -2.0, q[:], MUL, ADD)      # -2a + 3b - c
                stt(C1pe[:], a1[:], 3.0, b1[:], MUL, SUB)   # 3a - b
                stt(C1ae[:], a0[:], 3.0, b0[:], MUL, SUB)

                RP2 = R2e.rearrange("p k b c -> p (k b) c")
                RP1 = R1e.rearrange("p k b c -> p (k b) c")
                U4e, tme, t2e = et("U4e"), et("tme"), et("t2e")
                stt(U4e[:, :, 0:1], Z[:, :, 0:1], 4.0, RP2[:, :, 0:1], MUL, ADD)
                stt(U4e[:, :, 1:2], Z[:, :, 1:2], -4.0, RP2[:, :, 7:8], MUL, ADD)

                Scpe = epool.tile([128, 2, BPC, 8], f32)
                nc.scalar.copy(out=Scpe.rearrange("p k b c -> p (k b c)"),
                               in_=S1ef[:])
                SP = Scpe.rearrange("p k b c -> p (k b) c")

                stt(tme[:], E0[:, :, 0:8:7], 1.5, U4e[:], ADD, MUL)
                nc.vector.tensor_mul(t2e[:], SP[:, :, 0:8:7], RP1[:, :, 0:8:7])
                nc.vector.tensor_add(tme[:], tme[:], t2e[:])
                nc.vector.tensor_mul(C1ae[:], C1ae[:], C1pe[:])  # t3e in-place
                nc.vector.tensor_add(tme[:], tme[:], C1ae[:])
                rese = epool.tile([128, 2 * BPC, 2], f32)
                nc.scalar.activation(rese[:], tme[:], COPY,
                                     bias=-1.0, scale=-GAMMA)

            # ------------- main pipeline, 4 chunks of 4 images -------------
            if True:
                for c in range(CHUNKS):
                    b0c = c * BCH
                    XbC = pool.tile([128, 2, 2, BCH, P], bf16, tag="xbc",
                                    bufs=3)
                    if c == 0:
                        nc.sync.dma_start(out=XbC[:, 1:2],
                                          in_=xb[:, 1:2, :, b0c:b0c + BCH, :])
                        nc.sync.dma_start(out=XbC[:, 0:1],
                                          in_=xb[:, 0:1, :, b0c:b0c + BCH, :])
                    else:
                        nc.sync.dma_start(out=XbC[:],
                                          in_=xb[:, :, :, b0c:b0c + BCH, :])
                    X0c = pool.tile([128, 2, BCH, P], f32r, tag="x0", bufs=3)
                    # X1 padded by one col each side so the +-1-shifted
                    # identity-matmul rhs views stay in bounds at full N
                    X1p = pool.tile([128, FCH + 2], f32r, tag="x1", bufs=3)
                    nc.sync.dma_start(
                        out=X1p[:, 1:FCH + 1].rearrange(
                            "p (k b j) -> p k b j", k=2, b=BCH),
                        in_=xin[:, 1, :, b0c:b0c + BCH, :])
                    nc.sync.dma_start(out=X0c[:], in_=xin[:, 0, :, b0c:b0c + BCH, :])
                    XbCf = XbC.rearrange("p c k b j -> p (c k b j)")
                    Xb0f = XbCf[:, 0:FCH]
                    Xb1f = XbCf[:, FCH:2 * FCH]
                    X0rf = X0c.rearrange("p k b j -> p (k b j)")
                    X1rf = X1p[:, 1:FCH + 1]
                    X0f = X0c.bitcast(f32).rearrange("p k b j -> p (k b j)")
                    X1pf = X1p.bitcast(f32)
                    X1f = X1pf[:, 1:FCH + 1]
                    # "sh" tiles hold values for column index t+1 at slot t,
                    # keeping every bf16 operand 4-byte aligned (2x mode).
                    C1p = pool.tile([128, FCH], bf16, tag="c1p", bufs=3)
                    C1a = pool.tile([128, FCH], bf16, tag="c1a", bufs=3)
                    scp = pool.tile([128, FCH], bf16, tag="scp", bufs=3)
                    rcp = pool.tile([128, FCH], bf16, tag="rcp", bufs=3)
                    t2b = pool.tile([128, FCH], bf16, tag="t2b", bufs=3)
                    t3b = pool.tile([128, FCH], bf16, tag="t3b", bufs=3)
                    tm = pool.tile([128, 2, BCH, P], f32, tag="tm", bufs=3)
                    tmf = tm.rearrange("p k b j -> p (k b j)")

                    # column stencils (interior; border cols handled above)
                    nc.vector.tensor_sub(C1p[:, 0:FCH - 2], Xb1f[:, 2:FCH],
                                         Xb1f[:, 0:FCH - 2])
                    nc.vector.tensor_sub(C1a[:, 0:FCH - 2], Xb0f[:, 2:FCH],
                                         Xb0f[:, 0:FCH - 2])
                    nc.vector.tensor_mul(t3b[:], C1a[:], C1p[:])

                    for m in range(2):
                        NBP = BCH // 2
                        R1s = [pp.tile([128, 2 * P], f32, name=f"r1_{c}_{m}_{bp}",
                                       tag="r1") for bp in range(NBP)]
                        S1s = [pp.tile([128, 2 * P], f32, name=f"s1_{c}_{m}_{bp}",
                                       tag="s1") for bp in range(NBP)]
                        U4s = [pp.tile([128, 2 * P], f32, name=f"r2_{c}_{m}_{bp}",
                                       tag="r2") for bp in range(NBP)]
                        for kb in range(2):
                            st, sp = kb == 0, kb == 1
                            isls = [slice(kb * (BCH * P) + bp * (2 * P),
                                          kb * (BCH * P) + (bp + 1) * (2 * P))
                                    for bp in range(NBP)]
                            for bp in range(NBP):
                                nc.tensor.matmul(R1s[bp][:], Wb(m * 2 + kb),
                                                 Xb1f[:, isls[bp]],
                                                 start=st, stop=sp)
                            for bp in range(NBP):
                                nc.tensor.matmul(S1s[bp][:], Wb(m * 2 + kb),
                                                 Xb0f[:, isls[bp]],
                                                 start=st, stop=sp)
                            for bp in range(NBP):
                                nc.tensor.matmul(U4s[bp][:], W(4 + m * 2 + kb),
                                                 X1rf[:, isls[bp]],
                                                 start=st, stop=False)
                        # column-neighbor sums via 4I with +-1-shifted rhs:
                        # U4 = W_R2@X1 + 4I@X1[+1] + 4I@X1[-1], all in PSUM
                        for bp in range(NBP):
                            lo = m * (BCH * P) + bp * (2 * P)
                            hi = lo + 2 * P
                            # right neighbor (col+1)
                            nc.tensor.matmul(U4s[bp][:], W(12),
                                             X1p[:, lo + 2:hi + 2],
                                             start=False, stop=False)
                            nc.tensor.matmul(U4s[bp][:], W(12),
                                             X1p[:, lo:hi],
                                             start=False, stop=True)
                        for bp in range(NBP):
                            lo = m * (BCH * P) + bp * (2 * P)
                            sl = slice(lo, lo + 2 * P)
                            # shifted bf16 evacuation: scp[t] = S1[t+1]
                            nc.scalar.copy(out=scp[:, lo:lo + 2 * P - 1],
                                           in_=S1s[bp][:, 1:2 * P])
                            nc.scalar.copy(out=rcp[:, lo:lo + 2 * P - 1],
                                           in_=R1s[bp][:, 1:2 * P])
                            # tm = (X0 + 1.5) * U4  (PSUM source)
                            stt(tmf[:, sl], X0f[:, sl], 1.5, U4s[bp][:],
                                ADD, MUL)

                    # gradient-product terms, all bf16 in the shifted
                    # layout: t2b = scp*rcp, += t3b, then one mixed add
                    nc.vector.tensor_mul(t2b[:], scp[:], rcp[:])
                    nc.vector.tensor_add(t2b[:], t2b[:], t3b[:])
                    nc.vector.tensor_add(tmf[:, 1:FCH], tmf[:, 1:FCH],
                                         t2b[:, 0:FCH - 1])

                    tmr = tm.rearrange("p k b j -> p (k b) j")
                    for m in range(2):
                        resv = tmr[:, m * BCH:(m + 1) * BCH, 1:P - 1]
                        nc.scalar.activation(resv, resv, COPY,
                                             bias=-1.0, scale=-GAMMA)
                        for e, j in ((0, 0), (1, P - 1)):
                            esrc = (rese.rearrange("p (k b) e -> p k b e", k=2)
                                    [:, m:m + 1, b0c:b0c + BCH, e:e + 1])
                            nc.scalar.copy(out=tm[:, m:m + 1, :, j:j + 1],
                                           in_=esrc)
                        nc.gpsimd.dma_start(
                            out=yout[:, m:m + 1, b0c:b0c + BCH, 0:P],
                            in_=tm[:, m:m + 1])

    nc.compile()
    return nc


def _get_program():
    if "nc" not in _cache:
        _cache["nc"] = _build_program()
        _cache["wts"] = _weights()
        import ml_dtypes
        _cache["ident"] = np.eye(128, dtype=ml_dtypes.bfloat16)
        D1 = np.zeros((P, P))
        for i in range(1, P - 1):
            D1[i, i - 1] = -1.0
            D1[i, i + 1] = 1.0
        D1[0, 0:3] = [-3.0, 4.0, -1.0]
        D1[P - 1, P - 3:P] = [1.0, -4.0, 3.0]
        wtbd = np.zeros((128, 4, 128), dtype=ml_dtypes.bfloat16)
        for m in range(2):
            for kb in range(2):
                wtbd[:, m * 2 + kb, :] = D1[m * 128:(m + 1) * 128,
                                            kb * 128:(kb + 1) * 128].T
        _cache["wtbd"] = wtbd
    return _cache["nc"], _cache["wts"], _cache["ident"]


def _shard_inputs(x0_pred):
    import ml_dtypes
    x = np.ascontiguousarray(np.asarray(x0_pred, dtype=np.float32))
    _, wts, ident = _get_program()
    wtbd = _cache["wtbd"]
    in_maps = []
    for i in range(NCORES):
        shard = x[i * BPC:(i + 1) * BPC]                      # [16,2,256,256]
        arr = shard.reshape(BPC, 2, 2, 128, P).transpose(3, 1, 2, 0, 4)
        arr = np.ascontiguousarray(arr)
        cols = [0, 1, 2, 3, P - 4, P - 3, P - 2, P - 1]
        xe = np.ascontiguousarray(arr[:, :, :, :, cols])
        xbi = arr.astype(ml_dtypes.bfloat16)
        in_maps.append({"xin": arr, "xe": xe, "xb": xbi, "wts": wts,
                        "ident": ident, "wtbd": wtbd})
    return in_maps


def _unshard(results):
    outs = []
    for i in range(NCORES):
        y = results[i]["yout"]                                # [128,2,16,256]
        outs.append(y.transpose(2, 1, 0, 3).reshape(BPC, 1, P, P))
    return np.ascontiguousarray(np.concatenate(outs, axis=0))


def _run(x0_pred, trace=False, tmpdir=None):
    from concourse.bass_utils import run_bass_kernel_spmd
    nc = _get_program()[0]
    in_maps = _shard_inputs(x0_pred)
    res = run_bass_kernel_spmd(nc, in_maps, list(range(NCORES)),
                               trace=trace, tmpdir=tmpdir)
    return _unshard(res.results), res


def kernel(x0_pred):
    out, _ = _run(x0_pred, trace=False)
    return out
